# revision 2
# baseline (speedup 1.0000x reference)
# Trainium2 Bass kernel for a 4-layer LSTM (B=32, T=2048, I=H=512),
# output = final cell states c_n (4, 32, 512).
#
# Strategy (mode "pipe", default):
#   8 cores = 4 layers x 2 batch halves. Each core runs ONE layer's
#   recurrence for a 16-sample batch half. Layer l+1 consumes layer l's
#   hidden-state sequence block-by-block (wavefront pipeline); blocks move
#   between cores with an AllGather over each 4-core chain per block.
# Mode "split" (fallback): batch-parallel only, each core runs all 4 layers
#   for 4 samples serially.
#
# Layout: everything gate-major. Weights pre-transposed/cast to bf16 on the
# host: lhsT tiles are (k_part=128, gate). Hidden state h kept as
# (128 h-dims, k-tile, batch) bf16 in SBUF; c state fp32. Input projections
# (xg) are precomputed per block with large efficient matmuls; the
# sequential recurrence streams Whh through the PE array each step
# (64 LDW+MM pairs, LDW-bound with automatic fast-weight-load for bf16).

import os
import numpy as np
import ml_dtypes

import concourse.bass as bass
import concourse.tile as tile
from concourse import bacc, mybir
from concourse.bass import ds
from concourse.bass_utils import run_bass_kernel_spmd
from concourse.expressions import smin, smax, s_not_equal

BF16 = mybir.dt.bfloat16
FP32 = mybir.dt.float32

# Problem constants (hardcoded per the contract)
B, T, I = 32, 2048, 512
H, L, G = 512, 4, 2048  # G = 4*H gates
KT = 4        # k tiles (512 / 128)
MT = 16       # m (gate) tiles (2048 / 128)
P = 128

MODE = os.environ.get("LSTM_MODE", "pipe")
T_OV = int(os.environ.get("LSTM_T_OVERRIDE", "0")) or T  # dev-only override

_cache = {}


def _bf16(a):
    return np.asarray(a, np.float32).astype(ml_dtypes.bfloat16)


# ---------------------------------------------------------------------------
# shared emitters
# ---------------------------------------------------------------------------

def _emit_phase_a(nc, pools, wih_sb, bias_sb, src_ap, src_roff, xg_dram, rows):
    """xg[g, r] = Wih.T @ inp + bias for `rows` rows starting at src_roff
    (ScalarValue or int offset into src_ap's last dim). Writes xg_dram
    (MT, 128, rows) fp32."""
    CH = 512
    nch = rows // CH
    for c in range(nch):
        inp = pools["mov"].tile([P, KT, CH], BF16, tag="mov")
        off = src_roff + c * CH if not isinstance(src_roff, int) else src_roff + c * CH
        nc.sync.dma_start(
            out=inp, in_=src_ap[:, :, ds(off, CH)].rearrange("a p c -> p a c")
        )
        for m in range(MT):
            ps = pools["psA"].tile([P, CH], FP32, tag="psA")
            for k in range(KT):
                nc.tensor.matmul(
                    ps,
                    lhsT=wih_sb[:, k, m * P:(m + 1) * P],
                    rhs=inp[:, k, :],
                    start=(k == 0),
                    stop=(k == KT - 1),
                )
            xs = pools["xgs"].tile([P, CH], FP32, tag="xgs")
            nc.vector.tensor_scalar_add(xs, ps, bias_sb[:, m:m + 1])
            nc.sync.dma_start(out=xg_dram[m, :, c * CH:(c + 1) * CH], in_=xs)


def _emit_steps(nc, tc, pools, whh_sb, xg_dram, h_sb, c_sb, hseq_ap, hseq_roff,
                nsteps, Bc, U, hint):
    """The sequential recurrence: nsteps LSTM steps. Reads xg_dram
    (MT,128,nsteps*Bc) fp32; h_sb (128,KT,Bc) bf16 and c_sb (128,KT,Bc) fp32
    updated in place; writes h rows into hseq_ap[:, :, hseq_roff + s]."""
    rows_per_iter = U * Bc

    with tc.For_i(0, nsteps * Bc, rows_per_iter, hint_engines=hint) as s:
        xg_u = pools["xgu"].tile([P, MT, rows_per_iter], FP32, tag="xgu")
        nc.sync.dma_start(
            out=xg_u,
            in_=xg_dram[:, :, ds(s, rows_per_iter)].rearrange("m p c -> p m c"),
        )
        hfl = pools["hfl"].tile([P, KT, rows_per_iter], BF16, tag="hfl")
        for u in range(U):
            ps = pools["psB"].tile([P, MT, Bc], FP32, tag="psB")
            for m in range(MT):
                for k in range(KT):
                    nc.tensor.matmul(
                        ps[:, m, :],
                        lhsT=whh_sb[:, k, m * P:(m + 1) * P],
                        rhs=h_sb[:, k, :],
                        start=(k == 0),
                        stop=(k == KT - 1),
                    )
            z = pools["z"].tile([P, MT, Bc], FP32, tag="z")
            nc.vector.tensor_add(out=z, in0=ps, in1=xg_u[:, :, u * Bc:(u + 1) * Bc])
            gts = pools["g"].tile([P, MT, Bc], FP32, tag="g")
            nc.scalar.activation(gts[:, 0:8, :], z[:, 0:8, :],
                                 mybir.ActivationFunctionType.Sigmoid)
            nc.scalar.activation(gts[:, 8:12, :], z[:, 8:12, :],
                                 mybir.ActivationFunctionType.Tanh)
            nc.scalar.activation(gts[:, 12:16, :], z[:, 12:16, :],
                                 mybir.ActivationFunctionType.Sigmoid)
            t1 = pools["t1"].tile([P, KT, Bc], FP32, tag="t1")
            t2 = pools["t2"].tile([P, KT, Bc], FP32, tag="t2")
            nc.vector.tensor_mul(t1, gts[:, 4:8, :], c_sb)      # f * c
            nc.vector.tensor_mul(t2, gts[:, 0:4, :], gts[:, 8:12, :])  # i * g
            nc.vector.tensor_add(c_sb, t1, t2)
            tc_t = pools["tc"].tile([P, KT, Bc], FP32, tag="tc")
            nc.scalar.activation(tc_t, c_sb, mybir.ActivationFunctionType.Tanh)
            nc.vector.tensor_mul(h_sb, gts[:, 12:16, :], tc_t)  # o * tanh(c) -> bf16
            nc.vector.tensor_copy(out=hfl[:, :, u * Bc:(u + 1) * Bc], in_=h_sb)
        hout_off = hseq_roff + s if not isinstance(hseq_roff, int) else hseq_roff + s
        nc.sync.dma_start(
            out=hseq_ap[:, :, ds(hout_off, rows_per_iter)].rearrange("a p c -> p a c"),
            in_=hfl,
        )


def _make_pools(tc, ctx, Bc, U):
    pools = {}
    pools["mov"] = ctx.enter_context(tc.tile_pool(name="mov", bufs=3))
    pools["psA"] = ctx.enter_context(tc.tile_pool(name="psA", bufs=2, space="PSUM"))
    pools["xgs"] = ctx.enter_context(tc.tile_pool(name="xgs", bufs=3))
    pools["xgu"] = ctx.enter_context(tc.tile_pool(name="xgu", bufs=2))
    pools["hfl"] = ctx.enter_context(tc.tile_pool(name="hfl", bufs=2))
    pools["psB"] = ctx.enter_context(tc.tile_pool(name="psB", bufs=2, space="PSUM"))
    for nm in ("z", "g", "t1", "t2", "tc"):
        pools[nm] = ctx.enter_context(tc.tile_pool(name=nm, bufs=2))
    return pools


# ---------------------------------------------------------------------------
# mode "split": batch-parallel, all layers per core
# ---------------------------------------------------------------------------

def _build_split(Tl):
    Bc = B // 8  # 4
    U = 16
    RT = Tl * Bc
    nc = bacc.Bacc("TRN2", target_bir_lowering=False, debug=False, num_devices=8)
    xT = nc.dram_tensor("xT", [KT, P, RT], BF16, kind="ExternalInput").ap()
    wih = nc.dram_tensor("wihT", [L, KT, P, G], BF16, kind="ExternalInput").ap()
    whh = nc.dram_tensor("whhT", [L, KT, P, G], BF16, kind="ExternalInput").ap()
    bias = nc.dram_tensor("bias", [L, MT, P], FP32, kind="ExternalInput").ap()
    h0 = nc.dram_tensor("h0T", [L, KT, P, Bc], BF16, kind="ExternalInput").ap()
    c0 = nc.dram_tensor("c0T", [L, KT, P, Bc], FP32, kind="ExternalInput").ap()
    cout = nc.dram_tensor("cT", [L, KT, P, Bc], FP32, kind="ExternalOutput").ap()

    xg_d = nc.dram_tensor("xg", [MT, P, RT], FP32, kind="Internal").ap()
    hs_a = nc.dram_tensor("hseqA", [KT, P, RT], BF16, kind="Internal").ap()
    hs_b = nc.dram_tensor("hseqB", [KT, P, RT], BF16, kind="Internal").ap()

    from contextlib import ExitStack
    with tile.TileContext(nc) as tc, ExitStack() as ctx:
        pools = _make_pools(tc, ctx, Bc, U)
        singles = ctx.enter_context(tc.tile_pool(name="singles", bufs=1))
        wih_sb = singles.tile([P, KT, G], BF16, tag="wih")
        whh_sb = singles.tile([P, KT, G], BF16, tag="whh")
        bias_sb = singles.tile([P, MT], FP32, tag="bias")
        h_sb = singles.tile([P, KT, Bc], BF16, tag="h")
        c_sb = singles.tile([P, KT, Bc], FP32, tag="c")
        hint = (mybir.EngineType.PE, mybir.EngineType.DVE,
                mybir.EngineType.Activation, mybir.EngineType.SP)

        for l in range(L):
            nc.sync.dma_start(out=wih_sb, in_=wih[l].rearrange("a p g -> p a g"))
            nc.sync.dma_start(out=whh_sb, in_=whh[l].rearrange("a p g -> p a g"))
            nc.sync.dma_start(out=bias_sb, in_=bias[l].rearrange("m p -> p m"))
            nc.sync.dma_start(out=h_sb, in_=h0[l].rearrange("a p b -> p a b"))
            nc.sync.dma_start(out=c_sb, in_=c0[l].rearrange("a p b -> p a b"))
            src = xT if l == 0 else (hs_a if l % 2 == 1 else hs_b)
            dst = hs_a if l % 2 == 0 else hs_b
            _emit_phase_a(nc, pools, wih_sb, bias_sb, src, 0, xg_d, RT)
            _emit_steps(nc, tc, pools, whh_sb, xg_d, h_sb, c_sb, dst, 0,
                        Tl, Bc, U, hint)
            nc.sync.dma_start(out=cout[l].rearrange("a p b -> p a b"), in_=c_sb)
    nc.compile()
    return nc


def _prep_split(x, h0, c0, w_ih, w_hh, b_ih, b_hh, Tl):
    Bc = B // 8
    ins = []
    wihT = np.ascontiguousarray(
        _bf16(w_ih).transpose(0, 2, 1).reshape(L, KT, P, G))
    whhT = np.ascontiguousarray(
        _bf16(w_hh).transpose(0, 2, 1).reshape(L, KT, P, G))
    bias = np.ascontiguousarray(
        (np.asarray(b_ih, np.float32) + np.asarray(b_hh, np.float32))
        .reshape(L, MT, P))
    for c in range(8):
        bs = slice(c * Bc, (c + 1) * Bc)
        # xT[kt, p, t*Bc + b] = x[b, t, kt*128+p]
        xc = np.asarray(x[bs, :Tl, :], np.float32)  # (Bc, Tl, I)
        xT = np.ascontiguousarray(
            _bf16(xc).transpose(2, 1, 0).reshape(KT, P, Tl * Bc))
        h0T = np.ascontiguousarray(
            _bf16(h0[:, bs, :]).transpose(0, 2, 1).reshape(L, KT, P, Bc))
        c0T = np.ascontiguousarray(
            np.asarray(c0[:, bs, :], np.float32).transpose(0, 2, 1)
            .reshape(L, KT, P, Bc))
        ins.append({"xT": xT, "wihT": wihT, "whhT": whhT, "bias": bias,
                    "h0T": h0T, "c0T": c0T})
    return ins


def _post_split(results):
    Bc = B // 8
    out = np.zeros((L, B, H), np.float32)
    for c, r in enumerate(results):
        ct = r["cT"]  # (L, KT, P, Bc)
        out[:, c * Bc:(c + 1) * Bc, :] = ct.reshape(L, H, Bc).transpose(0, 2, 1)
    return out


# ---------------------------------------------------------------------------
# mode "pipe": layer pipeline x batch halves
# ---------------------------------------------------------------------------

def _build_pipe(Tl, BLK):
    Bc = B // 2  # 16
    U = 16
    NB = Tl // BLK
    RB = BLK * Bc          # rows per block
    RT = Tl * Bc
    LAG = L - 1
    nc = bacc.Bacc("TRN2", target_bir_lowering=False, debug=False, num_devices=8)
    xT = nc.dram_tensor("xT", [KT, P, RT], BF16, kind="ExternalInput").ap()
    wih = nc.dram_tensor("wihT", [KT, P, G], BF16, kind="ExternalInput").ap()
    whh = nc.dram_tensor("whhT", [KT, P, G], BF16, kind="ExternalInput").ap()
    bias = nc.dram_tensor("bias", [MT, P], FP32, kind="ExternalInput").ap()
    h0 = nc.dram_tensor("h0T", [KT, P, Bc], BF16, kind="ExternalInput").ap()
    c0 = nc.dram_tensor("c0T", [KT, P, Bc], FP32, kind="ExternalInput").ap()
    # ctrl scalars: [l, l*RB, prev_slot]
    ctrl = nc.dram_tensor("ctrl", [1, 4], mybir.dt.uint32, kind="ExternalInput").ap()
    cout = nc.dram_tensor("cT", [KT, P, Bc], FP32, kind="ExternalOutput").ap()

    xg_d = nc.dram_tensor("xg", [MT, P, RB], FP32, kind="Internal").ap()
    sendb = nc.dram_tensor("sendb", [KT, P, RB], BF16, kind="Internal").ap()
    gath = nc.dram_tensor("gath", [4, KT, P, RB], BF16, kind="Internal").ap()

    from contextlib import ExitStack
    with tile.TileContext(nc) as tc, ExitStack() as ctx:
        pools = _make_pools(tc, ctx, Bc, U)
        singles = ctx.enter_context(tc.tile_pool(name="singles", bufs=1))
        wih_sb = singles.tile([P, KT, G], BF16, tag="wih")
        whh_sb = singles.tile([P, KT, G], BF16, tag="whh")
        bias_sb = singles.tile([P, MT], FP32, tag="bias")
        h_sb = singles.tile([P, KT, Bc], BF16, tag="h")
        c_sb = singles.tile([P, KT, Bc], FP32, tag="c")
        hint = (mybir.EngineType.PE, mybir.EngineType.DVE,
                mybir.EngineType.Activation, mybir.EngineType.SP)

        nc.sync.dma_start(out=wih_sb, in_=wih.rearrange("a p g -> p a g"))
        nc.sync.dma_start(out=whh_sb, in_=whh.rearrange("a p g -> p a g"))
        nc.sync.dma_start(out=bias_sb, in_=bias.rearrange("m p -> p m"))

        eng = nc.sync
        l_sv = _load_ctrl(nc, eng, ctrl, 0, 3)
        lrb_sv = _load_ctrl(nc, eng, ctrl, 1, LAG * RB)
        pslot_sv = _load_ctrl(nc, eng, ctrl, 2, 3)

        for j in range(NB + LAG):
            # block index this core works on: clamp(j - l, 0, NB-1) * RB
            roff = smax(smin(j * RB - lrb_sv, (NB - 1) * RB), 0)
            # exchange h blocks (contents of sendb were written in iter j-1)
            nc.gpsimd.collective_compute(
                kind="AllGather", op=mybir.AluOpType.bypass,
                replica_groups=[[0, 1, 2, 3], [4, 5, 6, 7]],
                ins=[sendb], outs=[gath],
            )
            # receive predecessor's block into my input sequence (l>0 only)
            nc.sync.dma_start(
                out=xT[:, :, ds(roff, RB)],
                in_=gath[ds(pslot_sv, 1), :, :, :].rearrange("o a p c -> (o a) p c"),
                cond=s_not_equal(l_sv, 0),
            )
            # state init on my first real block
            is_first = 1 - s_not_equal(l_sv, j)
            nc.sync.dma_start(out=h_sb, in_=h0.rearrange("a p b -> p a b"),
                              cond=is_first)
            nc.sync.dma_start(out=c_sb, in_=c0.rearrange("a p b -> p a b"),
                              cond=is_first)
            _emit_phase_a(nc, pools, wih_sb, bias_sb, xT, roff, xg_d, RB)
            _emit_steps(nc, tc, pools, whh_sb, xg_d, h_sb, c_sb, sendb, 0,
                        BLK, Bc, U, hint)
            # write final c on my last real block
            is_last = 1 - s_not_equal(l_sv, j - NB + 1)
            nc.sync.dma_start(out=cout.rearrange("a p b -> p a b"), in_=c_sb,
                              cond=is_last)
    nc.compile()
    return nc


def _load_ctrl(nc, eng, ctrl, idx, max_val):
    reg = eng.alloc_register(f"ctrl{idx}")
    eng.reg_load(reg, ctrl[0:1, idx:idx + 1])
    return eng.snap(reg, donate=True, min_val=0, max_val=max_val)


def _prep_pipe(x, h0, c0, w_ih, w_hh, b_ih, b_hh, Tl, BLK):
    Bc = B // 2
    RB = BLK * Bc
    bias_all = (np.asarray(b_ih, np.float32) + np.asarray(b_hh, np.float32))
    wihT = np.ascontiguousarray(_bf16(w_ih).transpose(0, 2, 1).reshape(L, KT, P, G))
    whhT = np.ascontiguousarray(_bf16(w_hh).transpose(0, 2, 1).reshape(L, KT, P, G))
    ins = []
    for c in range(8):
        half, l = c // 4, c % 4
        bs = slice(half * Bc, (half + 1) * Bc)
        xc = np.asarray(x[bs, :Tl, :], np.float32)
        xT = np.ascontiguousarray(_bf16(xc).transpose(2, 1, 0).reshape(KT, P, Tl * Bc))
        h0T = np.ascontiguousarray(_bf16(h0[l, bs, :]).T.reshape(KT, P, Bc))
        c0T = np.ascontiguousarray(
            np.asarray(c0[l, bs, :], np.float32).T.reshape(KT, P, Bc))
        ctrl = np.array([[l, l * RB, (l + 3) % 4, 0]], np.uint32)
        ins.append({"xT": xT, "wihT": wihT[l], "whhT": whhT[l],
                    "bias": bias_all[l].reshape(MT, P), "h0T": h0T, "c0T": c0T,
                    "ctrl": ctrl})
    return ins


def _post_pipe(results):
    Bc = B // 2
    out = np.zeros((L, B, H), np.float32)
    for c, r in enumerate(results):
        half, l = c // 4, c % 4
        ct = r["cT"]  # (KT, P, Bc)
        out[l, half * Bc:(half + 1) * Bc, :] = ct.reshape(H, Bc).T
    return out


# ---------------------------------------------------------------------------

def _get_built(mode, Tl):
    key = (mode, Tl)
    if key not in _cache:
        if mode == "split":
            _cache[key] = _build_split(Tl)
        else:
            BLK = int(os.environ.get("LSTM_BLK", "128"))
            _cache[key] = _build_pipe(Tl, BLK)
    return _cache[key]


def kernel(x, h0, c0, w_ih, w_hh, b_ih, b_hh):
    Tl = min(T_OV, np.asarray(x).shape[1])
    nc = _get_built(MODE, Tl)
    if MODE == "split":
        ins = _prep_split(x, h0, c0, w_ih, w_hh, b_ih, b_hh, Tl)
    else:
        BLK = int(os.environ.get("LSTM_BLK", "128"))
        ins = _prep_pipe(x, h0, c0, w_ih, w_hh, b_ih, b_hh, Tl, BLK)
    res = run_bass_kernel_spmd(nc, ins, core_ids=list(range(8)))
    out = _post_split(res.results) if MODE == "split" else _post_pipe(res.results)
    return out


# revision 9
# speedup vs baseline: 250.9733x; 250.9733x over previous
# Trainium2 Bass kernel for a 4-layer LSTM (B=32, T=2048, I=H=512),
# output = final cell states c_n (4, 32, 512).
#
# Strategy (mode "pipe", default):
#   8 cores = 4 layers x 2 batch halves. Each core runs ONE layer's
#   recurrence for a 16-sample batch half. Layer l+1 consumes layer l's
#   hidden-state sequence block-by-block (wavefront pipeline); blocks move
#   between cores with an AllGather over each 4-core chain per block.
# Mode "split" (fallback): batch-parallel only, each core runs all 4 layers
#   for 4 samples serially.
#
# Layout: everything gate-major. Weights pre-transposed/cast to bf16 on the
# host: lhsT tiles are (k_part=128, gate). Hidden state h kept as
# (128 h-dims, k-tile, batch) bf16 in SBUF; c state fp32. Input projections
# (xg) are precomputed per block with large efficient matmuls; the
# sequential recurrence streams Whh through the PE array each step
# (64 LDW+MM pairs, LDW-bound with automatic fast-weight-load for bf16).

import os
import numpy as np
import ml_dtypes

import concourse.bass as bass
import concourse.tile as tile
from concourse import bacc, mybir
from concourse.bass import ds
from concourse.bass_utils import run_bass_kernel_spmd
from concourse.expressions import smin, smax, s_not_equal

BF16 = mybir.dt.bfloat16
FP32 = mybir.dt.float32

# Problem constants (hardcoded per the contract)
B, T, I = 32, 2048, 512
H, L, G = 512, 4, 2048  # G = 4*H gates
KT = 4        # k tiles (512 / 128)
MT = 16       # m (gate) tiles (2048 / 128)
P = 128

MODE = os.environ.get("LSTM_MODE", "pipe")
T_OV = int(os.environ.get("LSTM_T_OVERRIDE", "0")) or T  # dev-only override
U_STEPS = int(os.environ.get("LSTM_U", "16"))
NO_CC = bool(int(os.environ.get("LSTM_NO_CC", "0")))  # timing diagnostic only
STAGGER = bool(int(os.environ.get("LSTM_STAGGER", "0")))
FAKE_STEPS = int(os.environ.get("LSTM_FAKE_STEPS", "-1"))  # diagnostic only
NO_PHA = bool(int(os.environ.get("LSTM_NO_PHA", "0")))  # diagnostic only
# gate-tile order in the fused weight layout: i,f,o,g so the three sigmoid
# regions are contiguous (2 ACT calls instead of 3)
GORDER = (0, 1, 3, 2)  # block g <-> o swap applied to (i,f,g,o) weight rows

_cache = {}


def _bf16(a):
    return np.asarray(a, np.float32).astype(ml_dtypes.bfloat16)


def _perm_gates(w):
    """Reorder gate blocks (i,f,g,o) -> (i,f,o,g) along dim 1 of (L, 4H, ...)."""
    w = np.asarray(w)
    blocks = w.reshape(w.shape[0], 4, H, *w.shape[2:])
    return np.ascontiguousarray(blocks[:, GORDER].reshape(w.shape))


# ---------------------------------------------------------------------------
# shared emitters
# ---------------------------------------------------------------------------

def _emit_phase_a(nc, pools, wih_sb, bias_sb, src_ap, src_roff, xg_dram, rows):
    """xg[g, r] = Wih.T @ inp + bias for `rows` rows starting at src_roff
    (ScalarValue or int offset into src_ap's last dim). Writes xg_dram
    (MT, 128, rows) fp32."""
    CH = 512
    nch = rows // CH
    for c in range(nch):
        inp = pools["mov"].tile([P, KT, CH], BF16, tag="mov")
        off = src_roff + c * CH if not isinstance(src_roff, int) else src_roff + c * CH
        nc.sync.dma_start(
            out=inp, in_=src_ap[:, :, ds(off, CH)].rearrange("a p c -> p a c")
        )
        for m in range(MT):
            ps = pools["psA"].tile([P, CH], FP32, tag="psA")
            for k in range(KT):
                nc.tensor.matmul(
                    ps,
                    lhsT=wih_sb[:, k, m * P:(m + 1) * P],
                    rhs=inp[:, k, :],
                    start=(k == 0),
                    stop=(k == KT - 1),
                )
            xs = pools["xgs"].tile([P, CH], FP32, tag="xgs")
            nc.vector.tensor_scalar_add(xs, ps, bias_sb[:, m:m + 1])
            nc.sync.dma_start(out=xg_dram[m, :, c * CH:(c + 1) * CH], in_=xs)


def _emit_steps(nc, tc, pools, whh_sb, xg_dram, h_sb, c_sb, hseq_ap, hseq_roff,
                nsteps, Bc, U, hint):
    """The sequential recurrence: nsteps LSTM steps. Reads xg_dram
    (MT,128,nsteps*Bc) fp32; h_sb (128,KT,Bc) bf16 and c_sb (128,KT,Bc) fp32
    updated in place; writes h rows into hseq_ap[:, :, hseq_roff + s]."""
    rows_per_iter = U * Bc

    with tc.For_i(0, nsteps * Bc, rows_per_iter, hint_engines=hint,
                  staggered_reset=STAGGER) as s:
        xg_u = pools["xgu"].tile([P, MT, rows_per_iter], FP32, tag="xgu")
        nc.sync.dma_start(
            out=xg_u,
            in_=xg_dram[:, :, ds(s, rows_per_iter)].rearrange("m p c -> p m c"),
        )
        hfl = pools["hfl"].tile([P, KT, rows_per_iter], BF16, tag="hfl")
        for u in range(U):
            ps = pools["psB"].tile([P, MT, Bc], FP32, tag="psB")
            for m in range(MT):
                for k in range(KT):
                    nc.tensor.matmul(
                        ps[:, m, :],
                        lhsT=whh_sb[:, k, m * P:(m + 1) * P],
                        rhs=h_sb[:, k, :],
                        start=(k == 0),
                        stop=(k == KT - 1),
                    )
            z = pools["z"].tile([P, MT, Bc], FP32, tag="z")
            nc.vector.tensor_add(out=z, in0=ps, in1=xg_u[:, :, u * Bc:(u + 1) * Bc])
            # weight rows are pre-permuted to (i, f, o, g) on the host
            gts = pools["g"].tile([P, MT, Bc], FP32, tag="g")
            nc.scalar.activation(gts[:, 0:12, :], z[:, 0:12, :],
                                 mybir.ActivationFunctionType.Sigmoid)
            nc.scalar.activation(gts[:, 12:16, :], z[:, 12:16, :],
                                 mybir.ActivationFunctionType.Tanh)
            t1 = pools["t1"].tile([P, KT, Bc], FP32, tag="t1")
            t2 = pools["t2"].tile([P, KT, Bc], FP32, tag="t2")
            nc.vector.tensor_mul(t1, gts[:, 4:8, :], c_sb)      # f * c
            nc.vector.tensor_mul(t2, gts[:, 0:4, :], gts[:, 12:16, :])  # i * g
            nc.vector.tensor_add(c_sb, t1, t2)
            tc_t = pools["tc"].tile([P, KT, Bc], FP32, tag="tc")
            nc.scalar.activation(tc_t, c_sb, mybir.ActivationFunctionType.Tanh)
            nc.vector.tensor_mul(h_sb, gts[:, 8:12, :], tc_t)  # o * tanh(c) -> bf16
            nc.vector.tensor_copy(out=hfl[:, :, u * Bc:(u + 1) * Bc], in_=h_sb)
        hout_off = hseq_roff + s if not isinstance(hseq_roff, int) else hseq_roff + s
        nc.sync.dma_start(
            out=hseq_ap[:, :, ds(hout_off, rows_per_iter)].rearrange("a p c -> p a c"),
            in_=hfl,
        )


def _make_pools(tc, ctx, Bc, U):
    pools = {}
    pools["mov"] = ctx.enter_context(tc.tile_pool(name="mov", bufs=3))
    pools["psA"] = ctx.enter_context(tc.tile_pool(name="psA", bufs=2, space="PSUM"))
    pools["xgs"] = ctx.enter_context(tc.tile_pool(name="xgs", bufs=3))
    pools["xgu"] = ctx.enter_context(tc.tile_pool(name="xgu", bufs=2))
    pools["hfl"] = ctx.enter_context(tc.tile_pool(name="hfl", bufs=2))
    pools["psB"] = ctx.enter_context(tc.tile_pool(name="psB", bufs=2, space="PSUM"))
    for nm in ("z", "g", "t1", "t2", "tc"):
        pools[nm] = ctx.enter_context(tc.tile_pool(name=nm, bufs=2))
    return pools


# ---------------------------------------------------------------------------
# mode "split": batch-parallel, all layers per core
# ---------------------------------------------------------------------------

def _build_split(Tl):
    Bc = B // 8  # 4
    U = U_STEPS
    RT = Tl * Bc
    nc = bacc.Bacc("TRN2", target_bir_lowering=False, debug=False, num_devices=8)
    xT = nc.dram_tensor("xT", [KT, P, RT], BF16, kind="ExternalInput").ap()
    wih = nc.dram_tensor("wihT", [L, KT, P, G], BF16, kind="ExternalInput").ap()
    whh = nc.dram_tensor("whhT", [L, KT, P, G], BF16, kind="ExternalInput").ap()
    bias = nc.dram_tensor("bias", [L, MT, P], FP32, kind="ExternalInput").ap()
    h0 = nc.dram_tensor("h0T", [L, KT, P, Bc], BF16, kind="ExternalInput").ap()
    c0 = nc.dram_tensor("c0T", [L, KT, P, Bc], FP32, kind="ExternalInput").ap()
    cout = nc.dram_tensor("cT", [L, KT, P, Bc], FP32, kind="ExternalOutput").ap()

    xg_d = nc.dram_tensor("xg", [MT, P, RT], FP32, kind="Internal").ap()
    hs_a = nc.dram_tensor("hseqA", [KT, P, RT], BF16, kind="Internal").ap()
    hs_b = nc.dram_tensor("hseqB", [KT, P, RT], BF16, kind="Internal").ap()

    from contextlib import ExitStack
    with tile.TileContext(nc) as tc, ExitStack() as ctx:
        pools = _make_pools(tc, ctx, Bc, U)
        singles = ctx.enter_context(tc.tile_pool(name="singles", bufs=1))
        wih_sb = singles.tile([P, KT, G], BF16, tag="wih")
        whh_sb = singles.tile([P, KT, G], BF16, tag="whh")
        bias_sb = singles.tile([P, MT], FP32, tag="bias")
        h_sb = singles.tile([P, KT, Bc], BF16, tag="h")
        c_sb = singles.tile([P, KT, Bc], FP32, tag="c")
        hint = (mybir.EngineType.PE, mybir.EngineType.DVE,
                mybir.EngineType.Activation, mybir.EngineType.SP)

        for l in range(L):
            nc.sync.dma_start(out=wih_sb, in_=wih[l].rearrange("a p g -> p a g"))
            nc.sync.dma_start(out=whh_sb, in_=whh[l].rearrange("a p g -> p a g"))
            nc.sync.dma_start(out=bias_sb, in_=bias[l].rearrange("m p -> p m"))
            nc.sync.dma_start(out=h_sb, in_=h0[l].rearrange("a p b -> p a b"))
            nc.sync.dma_start(out=c_sb, in_=c0[l].rearrange("a p b -> p a b"))
            src = xT if l == 0 else (hs_a if l % 2 == 1 else hs_b)
            dst = hs_a if l % 2 == 0 else hs_b
            _emit_phase_a(nc, pools, wih_sb, bias_sb, src, 0, xg_d, RT)
            _emit_steps(nc, tc, pools, whh_sb, xg_d, h_sb, c_sb, dst, 0,
                        Tl, Bc, U, hint)
            nc.sync.dma_start(out=cout[l].rearrange("a p b -> p a b"), in_=c_sb)
    nc.compile()
    return nc


def _prep_split(x, h0, c0, w_ih, w_hh, b_ih, b_hh, Tl):
    Bc = B // 8
    w_ih, w_hh = _perm_gates(w_ih), _perm_gates(w_hh)
    b_ih, b_hh = _perm_gates(b_ih[..., None])[..., 0], _perm_gates(b_hh[..., None])[..., 0]
    ins = []
    wihT = np.ascontiguousarray(
        _bf16(w_ih).transpose(0, 2, 1).reshape(L, KT, P, G))
    whhT = np.ascontiguousarray(
        _bf16(w_hh).transpose(0, 2, 1).reshape(L, KT, P, G))
    bias = np.ascontiguousarray(
        (np.asarray(b_ih, np.float32) + np.asarray(b_hh, np.float32))
        .reshape(L, MT, P))
    for c in range(8):
        bs = slice(c * Bc, (c + 1) * Bc)
        # xT[kt, p, t*Bc + b] = x[b, t, kt*128+p]
        xc = np.asarray(x[bs, :Tl, :], np.float32)  # (Bc, Tl, I)
        xT = np.ascontiguousarray(
            _bf16(xc).transpose(2, 1, 0).reshape(KT, P, Tl * Bc))
        h0T = np.ascontiguousarray(
            _bf16(h0[:, bs, :]).transpose(0, 2, 1).reshape(L, KT, P, Bc))
        c0T = np.ascontiguousarray(
            np.asarray(c0[:, bs, :], np.float32).transpose(0, 2, 1)
            .reshape(L, KT, P, Bc))
        ins.append({"xT": xT, "wihT": wihT, "whhT": whhT, "bias": bias,
                    "h0T": h0T, "c0T": c0T})
    return ins


def _post_split(results):
    Bc = B // 8
    out = np.zeros((L, B, H), np.float32)
    for c, r in enumerate(results):
        ct = r["cT"]  # (L, KT, P, Bc)
        out[:, c * Bc:(c + 1) * Bc, :] = ct.reshape(L, H, Bc).transpose(0, 2, 1)
    return out


# ---------------------------------------------------------------------------
# mode "pipe": layer pipeline x batch halves
# ---------------------------------------------------------------------------

def _build_pipe(Tl, BLK):
    Bc = B // 2  # 16
    U = U_STEPS
    NB = Tl // BLK
    RB = BLK * Bc          # rows per block
    RT = Tl * Bc
    LAG = L - 1
    nc = bacc.Bacc("TRN2", target_bir_lowering=False, debug=False, num_devices=8)
    xT = nc.dram_tensor("xT", [KT, P, RT], BF16, kind="ExternalInput").ap()
    wih = nc.dram_tensor("wihT", [KT, P, G], BF16, kind="ExternalInput").ap()
    whh = nc.dram_tensor("whhT", [KT, P, G], BF16, kind="ExternalInput").ap()
    bias = nc.dram_tensor("bias", [MT, P], FP32, kind="ExternalInput").ap()
    h0 = nc.dram_tensor("h0T", [KT, P, Bc], BF16, kind="ExternalInput").ap()
    c0 = nc.dram_tensor("c0T", [KT, P, Bc], FP32, kind="ExternalInput").ap()
    # ctrl scalars: [l, l*RB, prev_slot]
    ctrl = nc.dram_tensor("ctrl", [1, 4], mybir.dt.uint32, kind="ExternalInput").ap()
    cout = nc.dram_tensor("cT", [KT, P, Bc], FP32, kind="ExternalOutput").ap()

    xg_d = nc.dram_tensor("xg", [MT, P, RB], FP32, kind="Internal").ap()
    sendb = nc.dram_tensor("sendb", [KT, P, RB], BF16, kind="Internal").ap()
    gath = nc.dram_tensor("gath", [4, KT, P, RB], BF16, kind="Internal").ap()

    from contextlib import ExitStack
    with tile.TileContext(nc) as tc, ExitStack() as ctx:
        pools = _make_pools(tc, ctx, Bc, U)
        singles = ctx.enter_context(tc.tile_pool(name="singles", bufs=1))
        wih_sb = singles.tile([P, KT, G], BF16, tag="wih")
        whh_sb = singles.tile([P, KT, G], BF16, tag="whh")
        bias_sb = singles.tile([P, MT], FP32, tag="bias")
        h_sb = singles.tile([P, KT, Bc], BF16, tag="h")
        c_sb = singles.tile([P, KT, Bc], FP32, tag="c")
        hint = (mybir.EngineType.PE, mybir.EngineType.DVE,
                mybir.EngineType.Activation, mybir.EngineType.SP)

        nc.sync.dma_start(out=wih_sb, in_=wih.rearrange("a p g -> p a g"))
        nc.sync.dma_start(out=whh_sb, in_=whh.rearrange("a p g -> p a g"))
        nc.sync.dma_start(out=bias_sb, in_=bias.rearrange("m p -> p m"))

        eng = nc.sync
        l_sv = _load_ctrl(nc, eng, ctrl, 0, 3)
        lrb_sv = _load_ctrl(nc, eng, ctrl, 1, LAG * RB)
        pslot_sv = _load_ctrl(nc, eng, ctrl, 2, 3)

        for j in range(NB + LAG):
            # block index this core works on: clamp(j - l, 0, NB-1) * RB
            roff = smax(smin(j * RB - lrb_sv, (NB - 1) * RB), 0)
            # exchange h blocks (contents of sendb were written in iter j-1)
            if not NO_CC:
                nc.gpsimd.collective_compute(
                    kind="AllGather", op=mybir.AluOpType.bypass,
                    replica_groups=[[0, 1, 2, 3], [4, 5, 6, 7]],
                    ins=[sendb], outs=[gath],
                )
            # receive predecessor's block into my input sequence (l>0 only)
            nc.sync.dma_start(
                out=xT[:, :, ds(roff, RB)],
                in_=gath[ds(pslot_sv, 1), :, :, :].rearrange("o a p c -> (o a) p c"),
                cond=s_not_equal(l_sv, 0),
            )
            # state init on my first real block
            is_first = 1 - s_not_equal(l_sv, j)
            nc.sync.dma_start(out=h_sb, in_=h0.rearrange("a p b -> p a b"),
                              cond=is_first)
            nc.sync.dma_start(out=c_sb, in_=c0.rearrange("a p b -> p a b"),
                              cond=is_first)
            if not NO_PHA:
                _emit_phase_a(nc, pools, wih_sb, bias_sb, xT, roff, xg_d, RB)
            nst = BLK if FAKE_STEPS < 0 else FAKE_STEPS
            if nst:
                _emit_steps(nc, tc, pools, whh_sb, xg_d, h_sb, c_sb, sendb, 0,
                            nst, Bc, U, hint)
            # write final c on my last real block
            is_last = 1 - s_not_equal(l_sv, j - NB + 1)
            nc.sync.dma_start(out=cout.rearrange("a p b -> p a b"), in_=c_sb,
                              cond=is_last)
    nc.compile()
    return nc


def _load_ctrl(nc, eng, ctrl, idx, max_val):
    reg = eng.alloc_register(f"ctrl{idx}")
    eng.reg_load(reg, ctrl[0:1, idx:idx + 1])
    return eng.snap(reg, donate=True, min_val=0, max_val=max_val)


def _prep_pipe(x, h0, c0, w_ih, w_hh, b_ih, b_hh, Tl, BLK):
    Bc = B // 2
    w_ih, w_hh = _perm_gates(w_ih), _perm_gates(w_hh)
    b_ih, b_hh = _perm_gates(b_ih[..., None])[..., 0], _perm_gates(b_hh[..., None])[..., 0]
    RB = BLK * Bc
    bias_all = (np.asarray(b_ih, np.float32) + np.asarray(b_hh, np.float32))
    wihT = np.ascontiguousarray(_bf16(w_ih).transpose(0, 2, 1).reshape(L, KT, P, G))
    whhT = np.ascontiguousarray(_bf16(w_hh).transpose(0, 2, 1).reshape(L, KT, P, G))
    ins = []
    for c in range(8):
        half, l = c // 4, c % 4
        bs = slice(half * Bc, (half + 1) * Bc)
        xc = np.asarray(x[bs, :Tl, :], np.float32)
        xT = np.ascontiguousarray(_bf16(xc).transpose(2, 1, 0).reshape(KT, P, Tl * Bc))
        h0T = np.ascontiguousarray(_bf16(h0[l, bs, :]).T.reshape(KT, P, Bc))
        c0T = np.ascontiguousarray(
            np.asarray(c0[l, bs, :], np.float32).T.reshape(KT, P, Bc))
        ctrl = np.array([[l, l * RB, (l + 3) % 4, 0]], np.uint32)
        ins.append({"xT": xT, "wihT": wihT[l], "whhT": whhT[l],
                    "bias": bias_all[l].reshape(MT, P), "h0T": h0T, "c0T": c0T,
                    "ctrl": ctrl})
    return ins


def _post_pipe(results):
    Bc = B // 2
    out = np.zeros((L, B, H), np.float32)
    for c, r in enumerate(results):
        half, l = c // 4, c % 4
        ct = r["cT"]  # (KT, P, Bc)
        out[l, half * Bc:(half + 1) * Bc, :] = ct.reshape(H, Bc).T
    return out


# ---------------------------------------------------------------------------

def _get_built(mode, Tl):
    key = (mode, Tl)
    if key not in _cache:
        if mode == "split":
            _cache[key] = _build_split(Tl)
        else:
            BLK = int(os.environ.get("LSTM_BLK", "256"))
            _cache[key] = _build_pipe(Tl, BLK)
    return _cache[key]


def kernel(x, h0, c0, w_ih, w_hh, b_ih, b_hh):
    Tl = min(T_OV, np.asarray(x).shape[1])
    nc = _get_built(MODE, Tl)
    if MODE == "split":
        ins = _prep_split(x, h0, c0, w_ih, w_hh, b_ih, b_hh, Tl)
    else:
        BLK = int(os.environ.get("LSTM_BLK", "256"))
        ins = _prep_pipe(x, h0, c0, w_ih, w_hh, b_ih, b_hh, Tl, BLK)
    res = run_bass_kernel_spmd(nc, ins, core_ids=list(range(8)))
    out = _post_split(res.results) if MODE == "split" else _post_pipe(res.results)
    return out
